# revision 2
# baseline (speedup 1.0000x reference)
"""Self-contained Trainium2 Bass kernel for nn_Attention (B=8, N=1024, C=1024, H=16, D=64).

Sharding: data-parallel over batch B across the 8 NeuronCores (one batch element
per core, no collectives). Per-core program (all matmuls bf16, fp32 accumulate):

  - x is DMA'd per token tile and PE-transposed to xT [C, N] bf16 (cast free in
    the PSUM-drain copy), pipelining DMA/transpose/copy.
  - Weights are DMA'd fp32 into staging tiles and cast to bf16 on idle engines:
    w_v on ACT (idle during the projection phase), w_q/w_k strips and w_proj on
    DVE.
  - qT/kT are written bf16 by the DVE bias-add; scores sT[k,q] = kT.T @ qT run
    with K=64, packing the two heads of a pair onto disjoint PE row groups with
    adjacent emission so they execute concurrently (measured dt~4ns overlap).
  - p = exp(sT*scale) on ACT (bf16 out, no max-subtraction: scores are O(6)).
  - v carries an appended ones column, so oT_ext = v_ext.T @ p emits softmax
    denominators as row D. Normalization: DVE copies out of PSUM (denominator
    row to partition 0 - custom DVE ops ignore input base partition), then
    reciprocal_approx_fast + GPSIMD partition_broadcast + DVE multiply.
  - Emission interleaves next-pair q/k projection chunks and (pair 0) the v
    projection into the ACT-paced attention loop so the PE never idles.
  - PSUM: scores pool 2x[128,1024] (4 banks), accumulator pool 1x[128,1024]
    (2 banks), pv pool 1x[65,1024] (2 banks).
"""

import numpy as np

B = 8
N = 1024          # tokens
C = 1024          # model dim
H = 16            # heads
D = 64            # head dim
SCALE = D ** -0.5
NT = N // 128     # token tiles
CT = C // 128     # channel tiles
HP = H // 2       # head pairs

_CACHE: dict = {}


def _build_program(repeat: int = 1, max_phase: int = 3):
    import concourse.mybir as mybir
    import concourse.tile as tile
    from concourse import bacc
    from concourse.masks import make_identity
    import concourse.bass as bass

    F32 = mybir.dt.float32
    BF16 = mybir.dt.bfloat16
    AF = mybir.ActivationFunctionType

    nc = bacc.Bacc("TRN2", target_bir_lowering=False, debug=False, num_devices=B)

    x_ext = nc.declare_dram_parameter("x", [N, C], F32, isOutput=False)
    wqkv_ext = nc.declare_dram_parameter("w_qkv", [C, 3 * C], F32, isOutput=False)
    bqkv_ext = nc.declare_dram_parameter("b_qkv", [3 * C], F32, isOutput=False)
    wproj_ext = nc.declare_dram_parameter("w_proj", [C, C], F32, isOutput=False)
    bproj_ext = nc.declare_dram_parameter("b_proj", [C], F32, isOutput=False)
    out_ext = nc.declare_dram_parameter("out", [N, C], F32, isOutput=True)

    x_ap = x_ext.ap()
    wqkv_ap = wqkv_ext.ap()
    bqkv_ap = bqkv_ext.ap()
    wproj_ap = wproj_ext.ap()
    bproj_ap = bproj_ext.ap()
    out_ap = out_ext.ap()

    def bcast_row(src_1d_ap, parts):
        return bass.AP(
            tensor=src_1d_ap.tensor,
            offset=src_1d_ap.offset,
            ap=[[0, parts]] + [list(p) for p in src_1d_ap.ap],
        )

    with tile.TileContext(nc) as tc:
        # ---- persistent SBUF ----
        identity, _free_id = tc.tile([128, 128], F32, name="identity")
        make_identity(nc, identity)
        identity_bf, _free_idb = tc.tile([128, 128], BF16, name="identity_bf")
        nc.vector.tensor_copy(out=identity_bf, in_=identity)

        v_ext, _free_vext = tc.tile([128, NT, H, D + 1], BF16, name="v_ext")
        nc.vector.memset(v_ext[:, :, :, D : D + 1], 1.0)
        bq_pp, _free_bq = tc.tile([128, 2 * CT], F32, name="bq_pp")
        bv_bc, _free_bv = tc.tile([128, C], F32, name="bv_bc")
        bp_bc, _free_bp = tc.tile([128, C], F32, name="bp_bc")

        nc.sync.dma_start(
            out=bq_pp, in_=bqkv_ap[0 : 2 * C].rearrange("(t p) -> p t", p=128)
        )
        nc.gpsimd.dma_start(out=bv_bc, in_=bcast_row(bqkv_ap[2 * C : 3 * C], 128))
        nc.gpsimd.dma_start(out=bp_bc, in_=bcast_row(bproj_ap, 128))

        wqkv_t = wqkv_ap.rearrange("(kt p) c -> p kt c", p=128)
        wproj_src = wproj_ap.rearrange("(pj p) c -> p pj c", p=128)

        for rep in range(repeat):
            s = f"r{rep}_"

            xT, free_xT = tc.tile([128, CT, N], BF16, name=s + "xT")
            o_catT, free_ocat = tc.tile([128, CT, N], BF16, name=s + "o_catT")

            with (
                tc.tile_pool(name=s + "psA", bufs=2, space="PSUM") as psA,
                tc.tile_pool(name=s + "psB", bufs=1, space="PSUM") as psB,
                tc.tile_pool(name=s + "ps_o", bufs=1, space="PSUM") as ps_o_pool,
                tc.tile_pool(name=s + "wqk_st", bufs=2) as wqk_st_pool,
                tc.tile_pool(name=s + "wqk", bufs=2) as wqk_pool,
                tc.tile_pool(name=s + "stg", bufs=1) as stg_pool,
                tc.tile_pool(name=s + "qkT", bufs=2) as qkT_pool,
                tc.tile_pool(name=s + "pT", bufs=16) as pT_pool,
                tc.tile_pool(name=s + "o_raw", bufs=2) as o_raw_pool,
                tc.tile_pool(name=s + "l", bufs=2) as l_pool,
                tc.tile_pool(name=s + "y", bufs=2) as y_pool,
            ):

                def slotA(name):
                    return psA.tile([128, N], F32, name=s + name, tag="A")

                def slotB(name):
                    return psB.tile([128, N], F32, name=s + name, tag="Bacc")

                # q/k weight strips: DMA fp32 stage -> DVE cast -> bf16 strip
                def emit_wqk_dma(pj, queue):
                    stage = wqk_st_pool.tile(
                        [128, CT, 256], F32, name=f"{s}wqs{pj}", tag="wqk_st"
                    )
                    queue.dma_start(
                        out=stage[:, :, 0:128],
                        in_=wqkv_t[:, :, pj * 128 : (pj + 1) * 128],
                    )
                    queue.dma_start(
                        out=stage[:, :, 128:256],
                        in_=wqkv_t[:, :, C + pj * 128 : C + (pj + 1) * 128],
                    )
                    return stage

                def emit_wqk_cast(stage, pj):
                    strip = wqk_pool.tile(
                        [128, CT, 256], BF16, name=f"{s}wqk{pj}", tag="wqk"
                    )
                    nc.vector.tensor_copy(out=strip, in_=stage)
                    return strip

                # ---------- phase 0: x -> xT (per-tile DMA/transpose/copy) ----
                wv, free_wv = tc.tile([128, CT, C], BF16, name=s + "wv")
                with tc.tile_pool(name=s + "x_pool", bufs=3) as x_pool:
                    x_src = x_ap.rearrange("(i p) c -> p i c", p=128)
                    for i in range(NT):
                        x_i = x_pool.tile(
                            [128, C], F32, name=f"{s}x{i}", tag="x"
                        )
                        nc.sync.dma_start(out=x_i, in_=x_src[:, i, :])
                        ps_t = slotA(f"ps_t{i}")
                        for j in range(CT):
                            nc.tensor.matmul(
                                ps_t[:, j * 128 : (j + 1) * 128],
                                x_i[:, j * 128 : (j + 1) * 128],
                                identity,
                                is_transpose=True,
                            )
                        nc.vector.tensor_copy(
                            out=xT[:, :, i * 128 : (i + 1) * 128],
                            in_=ps_t.rearrange("p (j n) -> p j n", n=128),
                        )

                    # v weights: DMA fp32 halves -> ACT cast (ACT is idle here)
                    for ch in range(2):
                        csl = slice(2 * C + ch * 512, 2 * C + ch * 512 + 512)
                        wv_st = stg_pool.tile(
                            [128, CT, 512], F32, name=f"{s}wvst{ch}", tag="stg"
                        )
                        nc.gpsimd.dma_start(out=wv_st, in_=wqkv_t[:, :, csl])
                        nc.scalar.copy(
                            out=wv[:, :, ch * 512 : ch * 512 + 512], in_=wv_st
                        )

                def qk_mm_chunk(slot, strip, qk, ch):
                    nsl = slice(ch * 512, ch * 512 + 512)
                    for kt in range(CT):
                        nc.tensor.matmul(
                            slot[:, nsl],
                            strip[:, kt, qk * 128 : qk * 128 + 128],
                            xT[:, kt, nsl],
                            start=(kt == 0),
                            stop=(kt == CT - 1),
                        )

                def qk_drain(slot, qkT, pj, qk):
                    jj = qk * CT + pj
                    nc.vector.tensor_scalar_add(
                        out=qkT[:, qk, :], in0=slot, scalar1=bq_pp[:, jj : jj + 1]
                    )

                def emit_qk_full(pj, strip):
                    qkT = qkT_pool.tile(
                        [128, 2, N], BF16, name=f"{s}qkT{pj}", tag="qkT"
                    )
                    for qk in range(2):
                        slot = slotB(f"qk{pj}_{qk}")
                        for ch in range(2):
                            qk_mm_chunk(slot, strip, qk, ch)
                        qk_drain(slot, qkT, pj, qk)
                    return qkT

                def emit_v_chunk(m):
                    """Project v for token tile m (all 16 heads) -> v_ext[:, m]."""
                    slot = slotB(f"v{m}")
                    for ch in range(2):
                        nsl = slice(ch * 512, ch * 512 + 512)
                        for kt in range(CT):
                            nc.tensor.matmul(
                                slot[:, nsl],
                                xT[:, kt, m * 128 : (m + 1) * 128],
                                wv[:, kt, nsl],
                                start=(kt == 0),
                                stop=(kt == CT - 1),
                            )
                    nc.vector.tensor_add(
                        out=v_ext[:, m, :, 0:D],
                        in0=slot.rearrange("p (h d) -> p h d", d=D),
                        in1=bv_bc.rearrange("p (h d) -> p h d", d=D),
                    )

                def emit_pv(ps_o, h, kt, pT):
                    for ch in range(2):
                        nsl = slice(ch * 512, ch * 512 + 512)
                        nc.tensor.matmul(
                            ps_o[:, nsl],
                            v_ext[:, kt, h, :],
                            pT[:, nsl],
                            start=(kt == 0),
                            stop=(kt == NT - 1),
                        )

                def normalize(h, ps_o):
                    """ps_o [D+1, N] -> o_catT[hb:hb+64, pj, :] (normalized)."""
                    pj, hb = h // 2, (h % 2) * 64
                    o_raw = o_raw_pool.tile(
                        [D, N], F32, name=f"{s}o_raw{h}", tag="o_raw"
                    )
                    nc.vector.tensor_copy(out=o_raw, in_=ps_o[0:D, :])
                    l_den = l_pool.tile([1, N], F32, name=f"{s}l_den{h}", tag="l_den")
                    nc.vector.tensor_copy(out=l_den, in_=ps_o[D : D + 1, :])
                    l_inv = l_pool.tile([1, N], F32, name=f"{s}l_inv{h}", tag="l_inv")
                    nc.vector.reciprocal_approx_fast(out=l_inv, in_=l_den)
                    l_bc = l_pool.tile([D, N], F32, name=f"{s}l_bc{h}", tag="l_bc")
                    nc.gpsimd.partition_broadcast(l_bc, l_inv)
                    nc.vector.tensor_mul(
                        out=o_catT[hb : hb + 64, pj, :],
                        in0=o_raw,
                        in1=l_bc,
                    )

                # ---------- prologue ----------
                stage0 = emit_wqk_dma(0, nc.gpsimd)
                stage1 = emit_wqk_dma(1, nc.gpsimd)
                strip_next = emit_wqk_cast(stage0, 0)
                qkT_this = emit_qk_full(0, strip_next)
                strip_next = emit_wqk_cast(stage1, 1)
                wproj = None
                free_wproj = None

                # ---------- main pair loop ----------
                for p in range(HP):
                    h0, h1 = 2 * p, 2 * p + 1
                    ps_o = ps_o_pool.tile(
                        [D + 1, N], F32, name=f"{s}ps_o{h0}", tag="ps_o"
                    )
                    # next-pair qk slot (filled chunk-by-chunk inside kt loop)
                    qk_slot = None
                    qkT_next = None
                    if 0 < p < HP - 1:
                        qkT_next = qkT_pool.tile(
                            [128, 2, N], BF16, name=f"{s}qkT{p + 1}", tag="qkT"
                        )
                    pT_h1_tiles = []
                    pT_prev = None
                    pT1_prev = None
                    ps_o1_last = None
                    if p == HP - 1:
                        ps_o1_last = psB.tile(
                            [128, N], F32, name=f"{s}ps_o_last", tag="Bacc"
                        )[0 : D + 1, :]
                    for kt in range(NT):
                        ksl = slice(kt * 128, (kt + 1) * 128)
                        if p == 0:
                            emit_v_chunk(kt)
                        elif qkT_next is not None and kt % 2 == 1:
                            # one quarter of next pair's q/k projection
                            step = kt // 2  # 0..3 -> (q,ch0),(q,ch1),(k,ch0),(k,ch1)
                            qk, ch = step // 2, step % 2
                            if ch == 0:
                                qk_slot = slotB(f"qk{p + 1}_{qk}")
                            qk_mm_chunk(qk_slot, strip_next, qk, ch)
                            if ch == 1:
                                qk_drain(qk_slot, qkT_next, p + 1, qk)
                        # pv for even head, one iteration behind the exps
                        if pT_prev is not None:
                            emit_pv(ps_o, h0, kt - 1, pT_prev)
                            if p == HP - 1:
                                emit_pv(ps_o1_last, h1, kt - 1, pT1_prev)
                        sc0 = slotA(f"sc{h0}_{kt}")
                        sc1 = slotA(f"sc{h1}_{kt}")
                        for ch in range(2):
                            nsl = slice(ch * 512, ch * 512 + 512)
                            nc.tensor.matmul(
                                sc0[:, nsl],
                                qkT_this[0:64, 1, ksl],
                                qkT_this[0:64, 0, nsl],
                            )
                            nc.tensor.matmul(
                                sc1[:, nsl],
                                qkT_this[64:128, 1, ksl],
                                qkT_this[64:128, 0, nsl],
                            )
                        pT0 = pT_pool.tile(
                            [128, N], BF16, name=f"{s}pT{h0}_{kt}", tag="pT"
                        )
                        nc.scalar.activation(out=pT0, in_=sc0, func=AF.Exp, scale=SCALE)
                        pT1 = pT_pool.tile(
                            [128, N], BF16, name=f"{s}pT{h1}_{kt}", tag="pT"
                        )
                        nc.scalar.activation(out=pT1, in_=sc1, func=AF.Exp, scale=SCALE)
                        pT_h1_tiles.append(pT1)
                        pT_prev = pT0
                        pT1_prev = pT1
                    emit_pv(ps_o, h0, NT - 1, pT_prev)
                    if p == HP - 1:
                        emit_pv(ps_o1_last, h1, NT - 1, pT1_prev)
                    normalize(h0, ps_o)

                    if p == 0:
                        # pair 0 had v-projection as filler; qk(1) goes here
                        qkT_next = emit_qk_full(1, strip_next)
                    if p + 2 < HP:
                        stage = emit_wqk_dma(
                            p + 2, nc.sync if p % 2 == 0 else nc.gpsimd
                        )
                        strip_next = emit_wqk_cast(stage, p + 2)

                    if p == HP - 1:
                        normalize(h1, ps_o1_last)
                    else:
                        # pv for odd head from retained pT tiles
                        ps_o1 = ps_o_pool.tile(
                            [D + 1, N], F32, name=f"{s}ps_o{h1}", tag="ps_o"
                        )
                        for ch in range(2):
                            nsl = slice(ch * 512, ch * 512 + 512)
                            for kt in range(NT):
                                nc.tensor.matmul(
                                    ps_o1[:, nsl],
                                    v_ext[:, kt, h1, :],
                                    pT_h1_tiles[kt][:, nsl],
                                    start=(kt == 0),
                                    stop=(kt == NT - 1),
                                )
                        normalize(h1, ps_o1)

                    if p == 0:
                        free_wv()
                        wproj, free_wproj = tc.tile(
                            [128, CT, C], BF16, name=s + "wproj"
                        )
                    if p in (1, 2):
                        # w_proj: DMA fp32 half -> DVE cast (mid-attention)
                        ch = p - 1
                        wp_st = stg_pool.tile(
                            [128, CT, 512], F32, name=f"{s}wpst{ch}", tag="stg"
                        )
                        nc.gpsimd.dma_start(
                            out=wp_st, in_=wproj_src[:, :, ch * 512 : ch * 512 + 512]
                        )
                        nc.vector.tensor_copy(
                            out=wproj[:, :, ch * 512 : ch * 512 + 512], in_=wp_st
                        )

                    qkT_this = qkT_next

                # ---------- output projection ----------
                for m in range(NT):
                    slot = slotA(f"ym{m}") if m % 2 == 0 else slotB(f"ym{m}")
                    for ch in range(2):
                        nsl = slice(ch * 512, ch * 512 + 512)
                        for pj in range(CT):
                            nc.tensor.matmul(
                                slot[:, nsl],
                                o_catT[:, pj, m * 128 : (m + 1) * 128],
                                wproj[:, pj, nsl],
                                start=(pj == 0),
                                stop=(pj == CT - 1),
                            )
                    y_sb = y_pool.tile([128, C], F32, name=f"{s}y{m}", tag="y")
                    nc.vector.tensor_add(out=y_sb, in0=slot, in1=bp_bc)
                    nc.sync.dma_start(
                        out=out_ap[m * 128 : (m + 1) * 128, :], in_=y_sb
                    )

                free_wproj()

            free_ocat()
            free_xT()

        _free_bp()
        _free_bv()
        _free_bq()
        _free_vext()
        _free_idb()
        _free_id()

    nc.compile()
    return nc


def get_program(repeat: int = 1, max_phase: int = 3):
    key = ("nc", repeat, max_phase)
    if key not in _CACHE:
        _CACHE[key] = _build_program(repeat, max_phase)
    return _CACHE[key]


def _get_runner():
    """Persistent jitted SPMD executor (avoids re-tracing per kernel() call)."""
    if "runner" in _CACHE:
        return _CACHE["runner"]

    import jax
    from jax.sharding import Mesh, PartitionSpec
    from jax.experimental.shard_map import shard_map
    import concourse.mybir as mybir
    from concourse.bass2jax import (
        _bass_exec_p,
        install_neuronx_cc_hook,
        partition_id_tensor,
    )

    nc = get_program()
    install_neuronx_cc_hook()
    partition_name = nc.partition_id_tensor.name if nc.partition_id_tensor else None

    in_names, out_names, out_avals, zero_outs = [], [], [], []
    for alloc in nc.m.functions[0].allocations:
        if not isinstance(alloc, mybir.MemoryLocationSet):
            continue
        name = alloc.memorylocations[0].name
        if alloc.kind == "ExternalInput":
            if name != partition_name:
                in_names.append(name)
        elif alloc.kind == "ExternalOutput":
            shape = tuple(alloc.tensor_shape)
            dtype = mybir.dt.np(alloc.dtype)
            out_names.append(name)
            out_avals.append(jax.core.ShapedArray(shape, dtype))
            zero_outs.append(np.zeros((B * shape[0], *shape[1:]), dtype))
    n_params = len(in_names)
    in_names_all = list(in_names) + list(out_names)
    if partition_name is not None:
        in_names_all.append(partition_name)

    def _body(*args):
        operands = list(args)
        if partition_name is not None:
            operands.append(partition_id_tensor())
        return tuple(
            _bass_exec_p.bind(
                *operands,
                out_avals=tuple(out_avals),
                in_names=tuple(in_names_all),
                out_names=tuple(out_names),
                lowering_input_output_aliases=(),
                sim_require_finite=True,
                sim_require_nnan=True,
                nc=nc,
            )
        )

    devices = jax.devices()[:B]
    mesh = Mesh(np.asarray(devices), ("core",))
    n_outs = len(out_avals)
    sharded = jax.jit(
        shard_map(
            _body,
            mesh=mesh,
            in_specs=(PartitionSpec("core"),) * (n_params + n_outs),
            out_specs=(PartitionSpec("core"),) * n_outs,
            check_rep=False,
        ),
        keep_unused=True,
    )

    sharding = jax.sharding.NamedSharding(mesh, PartitionSpec("core"))
    dev_cache: dict = {}

    def _to_device(name, concat):
        import hashlib

        digest = hashlib.blake2b(concat.tobytes(), digest_size=16).digest()
        hit = dev_cache.get(name)
        if hit is not None and hit[0] == digest:
            return hit[1]
        arr = jax.device_put(concat, sharding)
        dev_cache[name] = (digest, arr)
        return arr

    def run(in_maps):
        concat_in = [
            _to_device(
                name,
                np.concatenate([np.asarray(m[name]) for m in in_maps], axis=0),
            )
            for name in in_names
        ]
        outs = sharded(*concat_in, *zero_outs)
        return {
            name: np.asarray(outs[i]).reshape(B, *out_avals[i].shape)
            for i, name in enumerate(out_names)
        }

    _CACHE["runner"] = run
    return run


def kernel(x, w_qkv, b_qkv, w_proj, b_proj):
    x = np.ascontiguousarray(np.asarray(x, dtype=np.float32))
    shared = {
        "w_qkv": np.ascontiguousarray(np.asarray(w_qkv, dtype=np.float32)),
        "b_qkv": np.ascontiguousarray(np.asarray(b_qkv, dtype=np.float32)),
        "w_proj": np.ascontiguousarray(np.asarray(w_proj, dtype=np.float32)),
        "b_proj": np.ascontiguousarray(np.asarray(b_proj, dtype=np.float32)),
    }
    in_maps = [{"x": x[b], **shared} for b in range(B)]
    run = _get_runner()
    res = run(in_maps)
    return res["out"].astype(np.float32)


# revision 4
# speedup vs baseline: 1.1107x; 1.1107x over previous
"""Self-contained Trainium2 Bass kernel for nn_Attention (B=8, N=1024, C=1024, H=16, D=64).

Sharding: data-parallel over batch B across the 8 NeuronCores (one batch element
per core, no collectives). Per-core program (all matmuls bf16, fp32 accumulate):

  - x is DMA'd per token tile and PE-transposed to xT [C, N] bf16 (cast free in
    the PSUM-drain copy), pipelining DMA/transpose/copy.
  - Weights are DMA'd fp32 into staging tiles and cast to bf16 on idle engines:
    w_v on ACT (idle during the projection phase), w_q/w_k strips and w_proj on
    DVE.
  - qT/kT are written bf16 by the DVE bias-add; scores sT[k,q] = kT.T @ qT run
    with K=64, packing the two heads of a pair onto disjoint PE row groups with
    adjacent emission so they execute concurrently (measured dt~4ns overlap).
  - p = exp(sT*scale) on ACT (bf16 out, no max-subtraction: scores are O(6)).
  - v carries an appended ones column, so oT_ext = v_ext.T @ p emits softmax
    denominators as row D. Normalization: DVE copies out of PSUM (denominator
    row to partition 0 - custom DVE ops ignore input base partition), then
    reciprocal_approx_fast + GPSIMD partition_broadcast + DVE multiply.
  - Emission interleaves next-pair q/k projection chunks and (pair 0) the v
    projection into the ACT-paced attention loop so the PE never idles.
  - PSUM: scores pool 2x[128,1024] (4 banks), accumulator pool 1x[128,1024]
    (2 banks), pv pool 1x[65,1024] (2 banks).
"""

import numpy as np

B = 8
N = 1024          # tokens
C = 1024          # model dim
H = 16            # heads
D = 64            # head dim
SCALE = D ** -0.5
NT = N // 128     # token tiles
CT = C // 128     # channel tiles
HP = H // 2       # head pairs

_CACHE: dict = {}


def _build_program(repeat: int = 1, max_phase: int = 3):
    import concourse.mybir as mybir
    import concourse.tile as tile
    from concourse import bacc
    from concourse.masks import make_identity
    import concourse.bass as bass

    F32 = mybir.dt.float32
    BF16 = mybir.dt.bfloat16
    AF = mybir.ActivationFunctionType

    nc = bacc.Bacc("TRN2", target_bir_lowering=False, debug=False, num_devices=B)

    x_ext = nc.declare_dram_parameter("x", [N, C], F32, isOutput=False)
    wqkv_ext = nc.declare_dram_parameter("w_qkv", [C, 3 * C], F32, isOutput=False)
    bqkv_ext = nc.declare_dram_parameter("b_qkv", [3 * C], F32, isOutput=False)
    wproj_ext = nc.declare_dram_parameter("w_proj", [C, C], F32, isOutput=False)
    bproj_ext = nc.declare_dram_parameter("b_proj", [C], F32, isOutput=False)
    out_ext = nc.declare_dram_parameter("out", [N, C], F32, isOutput=True)

    x_ap = x_ext.ap()
    wqkv_ap = wqkv_ext.ap()
    bqkv_ap = bqkv_ext.ap()
    wproj_ap = wproj_ext.ap()
    bproj_ap = bproj_ext.ap()
    out_ap = out_ext.ap()

    def bcast_row(src_1d_ap, parts):
        return bass.AP(
            tensor=src_1d_ap.tensor,
            offset=src_1d_ap.offset,
            ap=[[0, parts]] + [list(p) for p in src_1d_ap.ap],
        )

    with tile.TileContext(nc) as tc:
        # ---- persistent SBUF ----
        identity, _free_id = tc.tile([128, 128], F32, name="identity")
        make_identity(nc, identity)
        identity_bf, _free_idb = tc.tile([128, 128], BF16, name="identity_bf")
        nc.vector.tensor_copy(out=identity_bf, in_=identity)

        v_ext, _free_vext = tc.tile([128, NT, H, D + 1], BF16, name="v_ext")
        nc.vector.memset(v_ext[:, :, :, D : D + 1], 1.0)
        bq_pp, _free_bq = tc.tile([128, 2 * CT], F32, name="bq_pp")
        bv_bc, _free_bv = tc.tile([128, C], F32, name="bv_bc")
        bp_bc, _free_bp = tc.tile([128, C], F32, name="bp_bc")

        nc.sync.dma_start(
            out=bq_pp, in_=bqkv_ap[0 : 2 * C].rearrange("(t p) -> p t", p=128)
        )
        nc.gpsimd.dma_start(out=bv_bc, in_=bcast_row(bqkv_ap[2 * C : 3 * C], 128))
        nc.gpsimd.dma_start(out=bp_bc, in_=bcast_row(bproj_ap, 128))

        wqkv_t = wqkv_ap.rearrange("(kt p) c -> p kt c", p=128)
        wproj_src = wproj_ap.rearrange("(pj p) c -> p pj c", p=128)

        for rep in range(repeat):
            s = f"r{rep}_"

            xT, free_xT = tc.tile([128, CT, N], BF16, name=s + "xT")
            o_catT, free_ocat = tc.tile([128, CT, N], BF16, name=s + "o_catT")

            with (
                tc.tile_pool(name=s + "psA", bufs=2, space="PSUM") as psA,
                tc.tile_pool(name=s + "psB", bufs=1, space="PSUM") as psB,
                tc.tile_pool(name=s + "ps_o", bufs=1, space="PSUM") as ps_o_pool,
                tc.tile_pool(name=s + "wqk_st", bufs=2) as wqk_st_pool,
                tc.tile_pool(name=s + "wqk", bufs=2) as wqk_pool,
                tc.tile_pool(name=s + "stg", bufs=1) as stg_pool,
                tc.tile_pool(name=s + "qkT", bufs=2) as qkT_pool,
                tc.tile_pool(name=s + "pT", bufs=16) as pT_pool,
                tc.tile_pool(name=s + "o_raw", bufs=2) as o_raw_pool,
                tc.tile_pool(name=s + "l", bufs=2) as l_pool,
                tc.tile_pool(name=s + "y", bufs=2) as y_pool,
            ):

                def slotA(name):
                    return psA.tile([128, N], F32, name=s + name, tag="A")

                def slotB(name):
                    return psB.tile([128, N], F32, name=s + name, tag="Bacc")

                # q/k weight strips: DMA fp32 stage -> DVE cast -> bf16 strip
                def emit_wqk_dma(pj, queue):
                    stage = wqk_st_pool.tile(
                        [128, CT, 256], F32, name=f"{s}wqs{pj}", tag="wqk_st"
                    )
                    queue.dma_start(
                        out=stage[:, :, 0:128],
                        in_=wqkv_t[:, :, pj * 128 : (pj + 1) * 128],
                    )
                    queue.dma_start(
                        out=stage[:, :, 128:256],
                        in_=wqkv_t[:, :, C + pj * 128 : C + (pj + 1) * 128],
                    )
                    return stage

                def emit_wqk_cast(stage, pj):
                    strip = wqk_pool.tile(
                        [128, CT, 256], BF16, name=f"{s}wqk{pj}", tag="wqk"
                    )
                    nc.vector.tensor_copy(out=strip, in_=stage)
                    return strip

                # ---------- phase 0: x -> xT (per-tile DMA/transpose/copy) ----
                wv, free_wv = tc.tile([128, CT, C], BF16, name=s + "wv")
                with tc.tile_pool(name=s + "x_pool", bufs=3) as x_pool:
                    x_src = x_ap.rearrange("(i p) c -> p i c", p=128)
                    for i in range(NT):
                        x_i = x_pool.tile(
                            [128, C], F32, name=f"{s}x{i}", tag="x"
                        )
                        nc.sync.dma_start(out=x_i, in_=x_src[:, i, :])
                        ps_t = slotA(f"ps_t{i}")
                        for j in range(CT):
                            nc.tensor.matmul(
                                ps_t[:, j * 128 : (j + 1) * 128],
                                x_i[:, j * 128 : (j + 1) * 128],
                                identity,
                                is_transpose=True,
                            )
                        nc.vector.tensor_copy(
                            out=xT[:, :, i * 128 : (i + 1) * 128],
                            in_=ps_t.rearrange("p (j n) -> p j n", n=128),
                        )

                    # v weights: DMA fp32 halves -> ACT cast (ACT is idle here)
                    for ch in range(2):
                        csl = slice(2 * C + ch * 512, 2 * C + ch * 512 + 512)
                        wv_st = stg_pool.tile(
                            [128, CT, 512], F32, name=f"{s}wvst{ch}", tag="stg"
                        )
                        nc.gpsimd.dma_start(out=wv_st, in_=wqkv_t[:, :, csl])
                        nc.scalar.copy(
                            out=wv[:, :, ch * 512 : ch * 512 + 512], in_=wv_st
                        )

                def qk_mm_chunk(slot, strip, qk, ch):
                    nsl = slice(ch * 512, ch * 512 + 512)
                    for kt in range(CT):
                        nc.tensor.matmul(
                            slot[:, nsl],
                            strip[:, kt, qk * 128 : qk * 128 + 128],
                            xT[:, kt, nsl],
                            start=(kt == 0),
                            stop=(kt == CT - 1),
                        )

                def qk_drain(slot, qkT, pj, qk):
                    jj = qk * CT + pj
                    nc.vector.tensor_scalar_add(
                        out=qkT[:, qk, :], in0=slot, scalar1=bq_pp[:, jj : jj + 1]
                    )

                def emit_qk_full(pj, strip):
                    qkT = qkT_pool.tile(
                        [128, 2, N], BF16, name=f"{s}qkT{pj}", tag="qkT"
                    )
                    for qk in range(2):
                        slot = slotB(f"qk{pj}_{qk}")
                        for ch in range(2):
                            qk_mm_chunk(slot, strip, qk, ch)
                        qk_drain(slot, qkT, pj, qk)
                    return qkT

                def emit_v_chunk(m):
                    """Project v for token tile m (all 16 heads) -> v_ext[:, m]."""
                    slot = slotB(f"v{m}")
                    for ch in range(2):
                        nsl = slice(ch * 512, ch * 512 + 512)
                        for kt in range(CT):
                            nc.tensor.matmul(
                                slot[:, nsl],
                                xT[:, kt, m * 128 : (m + 1) * 128],
                                wv[:, kt, nsl],
                                start=(kt == 0),
                                stop=(kt == CT - 1),
                            )
                    nc.vector.tensor_add(
                        out=v_ext[:, m, :, 0:D],
                        in0=slot.rearrange("p (h d) -> p h d", d=D),
                        in1=bv_bc.rearrange("p (h d) -> p h d", d=D),
                    )

                def emit_pv(ps_o, h, kt, pT):
                    for ch in range(2):
                        nsl = slice(ch * 512, ch * 512 + 512)
                        nc.tensor.matmul(
                            ps_o[:, nsl],
                            v_ext[:, kt, h, :],
                            pT[:, nsl],
                            start=(kt == 0),
                            stop=(kt == NT - 1),
                        )

                def normalize(h, ps_o):
                    """ps_o [D+1, N] -> o_catT[hb:hb+64, pj, :] (normalized)."""
                    pj, hb = h // 2, (h % 2) * 64
                    o_raw = o_raw_pool.tile(
                        [D, N], F32, name=f"{s}o_raw{h}", tag="o_raw"
                    )
                    nc.vector.tensor_copy(out=o_raw, in_=ps_o[0:D, :])
                    l_den = l_pool.tile([1, N], F32, name=f"{s}l_den{h}", tag="l_den")
                    nc.vector.tensor_copy(out=l_den, in_=ps_o[D : D + 1, :])
                    l_inv = l_pool.tile([1, N], F32, name=f"{s}l_inv{h}", tag="l_inv")
                    nc.vector.reciprocal_approx_fast(out=l_inv, in_=l_den)
                    l_bc = l_pool.tile([D, N], F32, name=f"{s}l_bc{h}", tag="l_bc")
                    nc.gpsimd.partition_broadcast(l_bc, l_inv)
                    nc.vector.tensor_mul(
                        out=o_catT[hb : hb + 64, pj, :],
                        in0=o_raw,
                        in1=l_bc,
                    )

                def emit_sc_exp(p, kt, qkT):
                    """Scores + exps for both heads of pair p at token tile kt."""
                    h0, h1 = 2 * p, 2 * p + 1
                    ksl = slice(kt * 128, (kt + 1) * 128)
                    sc0 = slotA(f"sc{h0}_{kt}")
                    sc1 = slotA(f"sc{h1}_{kt}")
                    for ch in range(2):
                        nsl = slice(ch * 512, ch * 512 + 512)
                        nc.tensor.matmul(
                            sc0[:, nsl], qkT[0:64, 1, ksl], qkT[0:64, 0, nsl]
                        )
                        nc.tensor.matmul(
                            sc1[:, nsl], qkT[64:128, 1, ksl], qkT[64:128, 0, nsl]
                        )
                    pT0 = pT_pool.tile(
                        [128, N], BF16, name=f"{s}pT{h0}_{kt}", tag="pT"
                    )
                    nc.scalar.activation(out=pT0, in_=sc0, func=AF.Exp, scale=SCALE)
                    pT1 = pT_pool.tile(
                        [128, N], BF16, name=f"{s}pT{h1}_{kt}", tag="pT"
                    )
                    nc.scalar.activation(out=pT1, in_=sc1, func=AF.Exp, scale=SCALE)
                    return pT0, pT1

                # ---------- prologue ----------
                stage0 = emit_wqk_dma(0, nc.gpsimd)
                stage1 = emit_wqk_dma(1, nc.gpsimd)
                strip_next = emit_wqk_cast(stage0, 0)
                qkT_this = emit_qk_full(0, strip_next)
                strip_next = emit_wqk_cast(stage1, 1)
                emit_v_chunk(0)
                hoisted = emit_sc_exp(0, 0, qkT_this)
                wproj = None
                free_wproj = None

                # ---------- main pair loop ----------
                for p in range(HP):
                    h0, h1 = 2 * p, 2 * p + 1
                    ps_o = ps_o_pool.tile(
                        [D + 1, N], F32, name=f"{s}ps_o{h0}", tag="ps_o"
                    )
                    # next-pair qk slot (filled chunk-by-chunk inside kt loop)
                    qk_slot = None
                    qkT_next = None
                    if 0 < p < HP - 1:
                        qkT_next = qkT_pool.tile(
                            [128, 2, N], BF16, name=f"{s}qkT{p + 1}", tag="qkT"
                        )
                    pT_prev, pT1_0 = hoisted
                    pT_h1_tiles = [pT1_0]
                    pT1_prev = pT1_0
                    ps_o1_last = None
                    if p == HP - 1:
                        ps_o1_last = psB.tile(
                            [128, N], F32, name=f"{s}ps_o_last", tag="Bacc"
                        )[0 : D + 1, :]
                    for kt in range(1, NT):
                        if p == 0:
                            emit_v_chunk(kt)
                        elif qkT_next is not None and kt % 2 == 1:
                            # one quarter of next pair's q/k projection
                            step = kt // 2  # 0..3 -> (q,ch0),(q,ch1),(k,ch0),(k,ch1)
                            qk, ch = step // 2, step % 2
                            if ch == 0:
                                qk_slot = slotB(f"qk{p + 1}_{qk}")
                            qk_mm_chunk(qk_slot, strip_next, qk, ch)
                            if ch == 1:
                                qk_drain(qk_slot, qkT_next, p + 1, qk)
                        # pv for even head, one iteration behind the exps
                        emit_pv(ps_o, h0, kt - 1, pT_prev)
                        if p == HP - 1:
                            emit_pv(ps_o1_last, h1, kt - 1, pT1_prev)
                        pT0, pT1 = emit_sc_exp(p, kt, qkT_this)
                        pT_h1_tiles.append(pT1)
                        pT_prev = pT0
                        pT1_prev = pT1
                    emit_pv(ps_o, h0, NT - 1, pT_prev)
                    if p == HP - 1:
                        emit_pv(ps_o1_last, h1, NT - 1, pT1_prev)
                    normalize(h0, ps_o)

                    if p == 0:
                        # pair 0 had v-projection as filler; qk(1) goes here
                        qkT_next = emit_qk_full(1, strip_next)
                    if p + 2 < HP:
                        stage = emit_wqk_dma(
                            p + 2, nc.sync if p % 2 == 0 else nc.gpsimd
                        )
                        strip_next = emit_wqk_cast(stage, p + 2)

                    # hoist next pair's first scores+exps so ACT works
                    # through the pv_h1 boundary block below
                    if p + 1 < HP:
                        hoisted = emit_sc_exp(p + 1, 0, qkT_next)

                    if p == HP - 1:
                        normalize(h1, ps_o1_last)
                    else:
                        # pv for odd head from retained pT tiles
                        ps_o1 = ps_o_pool.tile(
                            [D + 1, N], F32, name=f"{s}ps_o{h1}", tag="ps_o"
                        )
                        for ch in range(2):
                            nsl = slice(ch * 512, ch * 512 + 512)
                            for kt in range(NT):
                                nc.tensor.matmul(
                                    ps_o1[:, nsl],
                                    v_ext[:, kt, h1, :],
                                    pT_h1_tiles[kt][:, nsl],
                                    start=(kt == 0),
                                    stop=(kt == NT - 1),
                                )
                        normalize(h1, ps_o1)

                    if p == 0:
                        free_wv()
                        wproj, free_wproj = tc.tile(
                            [128, CT, C], BF16, name=s + "wproj"
                        )
                    if p in (1, 2):
                        # w_proj: DMA fp32 half -> DVE cast (mid-attention)
                        ch = p - 1
                        wp_st = stg_pool.tile(
                            [128, CT, 512], F32, name=f"{s}wpst{ch}", tag="stg"
                        )
                        nc.gpsimd.dma_start(
                            out=wp_st, in_=wproj_src[:, :, ch * 512 : ch * 512 + 512]
                        )
                        nc.vector.tensor_copy(
                            out=wproj[:, :, ch * 512 : ch * 512 + 512], in_=wp_st
                        )

                    qkT_this = qkT_next

                # ---------- output projection ----------
                for m in range(NT):
                    slot = slotA(f"ym{m}") if m % 2 == 0 else slotB(f"ym{m}")
                    for ch in range(2):
                        nsl = slice(ch * 512, ch * 512 + 512)
                        for pj in range(CT):
                            nc.tensor.matmul(
                                slot[:, nsl],
                                o_catT[:, pj, m * 128 : (m + 1) * 128],
                                wproj[:, pj, nsl],
                                start=(pj == 0),
                                stop=(pj == CT - 1),
                            )
                    y_sb = y_pool.tile([128, C], F32, name=f"{s}y{m}", tag="y")
                    nc.vector.tensor_add(out=y_sb, in0=slot, in1=bp_bc)
                    nc.gpsimd.dma_start(
                        out=out_ap[m * 128 : (m + 1) * 128, :], in_=y_sb
                    )

                free_wproj()

            free_ocat()
            free_xT()

        _free_bp()
        _free_bv()
        _free_bq()
        _free_vext()
        _free_idb()
        _free_id()

    nc.compile()
    return nc


def get_program(repeat: int = 1, max_phase: int = 3):
    key = ("nc", repeat, max_phase)
    if key not in _CACHE:
        _CACHE[key] = _build_program(repeat, max_phase)
    return _CACHE[key]


def _get_runner():
    """Persistent jitted SPMD executor (avoids re-tracing per kernel() call)."""
    if "runner" in _CACHE:
        return _CACHE["runner"]

    import jax
    from jax.sharding import Mesh, PartitionSpec
    from jax.experimental.shard_map import shard_map
    import concourse.mybir as mybir
    from concourse.bass2jax import (
        _bass_exec_p,
        install_neuronx_cc_hook,
        partition_id_tensor,
    )

    nc = get_program()
    install_neuronx_cc_hook()
    partition_name = nc.partition_id_tensor.name if nc.partition_id_tensor else None

    in_names, out_names, out_avals, zero_outs = [], [], [], []
    for alloc in nc.m.functions[0].allocations:
        if not isinstance(alloc, mybir.MemoryLocationSet):
            continue
        name = alloc.memorylocations[0].name
        if alloc.kind == "ExternalInput":
            if name != partition_name:
                in_names.append(name)
        elif alloc.kind == "ExternalOutput":
            shape = tuple(alloc.tensor_shape)
            dtype = mybir.dt.np(alloc.dtype)
            out_names.append(name)
            out_avals.append(jax.core.ShapedArray(shape, dtype))
            zero_outs.append(np.zeros((B * shape[0], *shape[1:]), dtype))
    n_params = len(in_names)
    in_names_all = list(in_names) + list(out_names)
    if partition_name is not None:
        in_names_all.append(partition_name)

    def _body(*args):
        operands = list(args)
        if partition_name is not None:
            operands.append(partition_id_tensor())
        return tuple(
            _bass_exec_p.bind(
                *operands,
                out_avals=tuple(out_avals),
                in_names=tuple(in_names_all),
                out_names=tuple(out_names),
                lowering_input_output_aliases=(),
                sim_require_finite=True,
                sim_require_nnan=True,
                nc=nc,
            )
        )

    devices = jax.devices()[:B]
    mesh = Mesh(np.asarray(devices), ("core",))
    n_outs = len(out_avals)
    sharded = jax.jit(
        shard_map(
            _body,
            mesh=mesh,
            in_specs=(PartitionSpec("core"),) * (n_params + n_outs),
            out_specs=(PartitionSpec("core"),) * n_outs,
            check_rep=False,
        ),
        keep_unused=True,
    )

    sharding = jax.sharding.NamedSharding(mesh, PartitionSpec("core"))
    dev_cache: dict = {}

    def _to_device(name, concat):
        import hashlib

        digest = hashlib.blake2b(concat.tobytes(), digest_size=16).digest()
        hit = dev_cache.get(name)
        if hit is not None and hit[0] == digest:
            return hit[1]
        arr = jax.device_put(concat, sharding)
        dev_cache[name] = (digest, arr)
        return arr

    def run(in_maps):
        concat_in = [
            _to_device(
                name,
                np.concatenate([np.asarray(m[name]) for m in in_maps], axis=0),
            )
            for name in in_names
        ]
        outs = sharded(*concat_in, *zero_outs)
        return {
            name: np.asarray(outs[i]).reshape(B, *out_avals[i].shape)
            for i, name in enumerate(out_names)
        }

    _CACHE["runner"] = run
    return run


def kernel(x, w_qkv, b_qkv, w_proj, b_proj):
    x = np.ascontiguousarray(np.asarray(x, dtype=np.float32))
    shared = {
        "w_qkv": np.ascontiguousarray(np.asarray(w_qkv, dtype=np.float32)),
        "b_qkv": np.ascontiguousarray(np.asarray(b_qkv, dtype=np.float32)),
        "w_proj": np.ascontiguousarray(np.asarray(w_proj, dtype=np.float32)),
        "b_proj": np.ascontiguousarray(np.asarray(b_proj, dtype=np.float32)),
    }
    in_maps = [{"x": x[b], **shared} for b in range(B)]
    run = _get_runner()
    res = run(in_maps)
    return res["out"].astype(np.float32)


# revision 5
# speedup vs baseline: 1.2155x; 1.0943x over previous
"""Self-contained Trainium2 Bass kernel for nn_Attention (B=8, N=1024, C=1024, H=16, D=64).

Sharding: data-parallel over batch B across the 8 NeuronCores (one batch element
per core, no collectives). Per-core program (all matmuls bf16, fp32 accumulate):

  - x is DMA'd per token tile and PE-transposed to xT [C, N] bf16 (cast free in
    the PSUM-drain copy), pipelining DMA/transpose/copy.
  - Weights are DMA'd fp32 into staging tiles and cast to bf16 on idle engines:
    w_v on ACT (idle during the projection phase), w_q/w_k strips and w_proj on
    DVE.
  - qT/kT are written bf16 by the DVE bias-add; scores sT[k,q] = kT.T @ qT run
    with K=64, packing the two heads of a pair onto disjoint PE row groups with
    adjacent emission so they execute concurrently (measured dt~4ns overlap).
  - p = exp(sT*scale) on ACT (bf16 out, no max-subtraction: scores are O(6)).
  - v carries an appended ones column, so oT_ext = v_ext.T @ p emits softmax
    denominators as row D. Normalization: DVE copies out of PSUM (denominator
    row to partition 0 - custom DVE ops ignore input base partition), then
    reciprocal_approx_fast + GPSIMD partition_broadcast + DVE multiply.
  - Emission interleaves next-pair q/k projection chunks and (pair 0) the v
    projection into the ACT-paced attention loop so the PE never idles.
  - PSUM: scores pool 2x[128,1024] (4 banks), accumulator pool 1x[128,1024]
    (2 banks), pv pool 1x[65,1024] (2 banks).
"""

import numpy as np

B = 8
N = 1024          # tokens
C = 1024          # model dim
H = 16            # heads
D = 64            # head dim
SCALE = D ** -0.5
NT = N // 128     # token tiles
CT = C // 128     # channel tiles
HP = H // 2       # head pairs

_CACHE: dict = {}


def _build_program(repeat: int = 1, max_phase: int = 3):
    import concourse.mybir as mybir
    import concourse.tile as tile
    from concourse import bacc
    from concourse.masks import make_identity
    import concourse.bass as bass

    F32 = mybir.dt.float32
    BF16 = mybir.dt.bfloat16
    AF = mybir.ActivationFunctionType

    nc = bacc.Bacc("TRN2", target_bir_lowering=False, debug=False, num_devices=B)

    x_ext = nc.declare_dram_parameter("x", [N, C], F32, isOutput=False)
    wqkv_ext = nc.declare_dram_parameter("w_qkv", [C, 3 * C], F32, isOutput=False)
    bqkv_ext = nc.declare_dram_parameter("b_qkv", [3 * C], F32, isOutput=False)
    wproj_ext = nc.declare_dram_parameter("w_proj", [C, C], F32, isOutput=False)
    bproj_ext = nc.declare_dram_parameter("b_proj", [C], F32, isOutput=False)
    out_ext = nc.declare_dram_parameter("out", [N, C], F32, isOutput=True)

    x_ap = x_ext.ap()
    wqkv_ap = wqkv_ext.ap()
    bqkv_ap = bqkv_ext.ap()
    wproj_ap = wproj_ext.ap()
    bproj_ap = bproj_ext.ap()
    out_ap = out_ext.ap()

    def bcast_row(src_1d_ap, parts):
        return bass.AP(
            tensor=src_1d_ap.tensor,
            offset=src_1d_ap.offset,
            ap=[[0, parts]] + [list(p) for p in src_1d_ap.ap],
        )

    with tile.TileContext(nc) as tc:
        # ---- persistent SBUF ----
        identity, _free_id = tc.tile([128, 128], F32, name="identity")
        make_identity(nc, identity)
        identity_bf, _free_idb = tc.tile([128, 128], BF16, name="identity_bf")
        nc.vector.tensor_copy(out=identity_bf, in_=identity)

        v_ext, _free_vext = tc.tile([128, NT, H, D + 1], BF16, name="v_ext")
        nc.vector.memset(v_ext[:, :, :, D : D + 1], 1.0)
        bq_pp, _free_bq = tc.tile([128, 2 * CT], F32, name="bq_pp")
        bv_bc, _free_bv = tc.tile([128, C], F32, name="bv_bc")
        bp_bc, _free_bp = tc.tile([128, C], F32, name="bp_bc")

        nc.sync.dma_start(
            out=bq_pp, in_=bqkv_ap[0 : 2 * C].rearrange("(t p) -> p t", p=128)
        )
        nc.gpsimd.dma_start(out=bv_bc, in_=bcast_row(bqkv_ap[2 * C : 3 * C], 128))
        nc.gpsimd.dma_start(out=bp_bc, in_=bcast_row(bproj_ap, 128))

        wqkv_t = wqkv_ap.rearrange("(kt p) c -> p kt c", p=128)
        wproj_src = wproj_ap.rearrange("(pj p) c -> p pj c", p=128)

        for rep in range(repeat):
            s = f"r{rep}_"

            xT, free_xT = tc.tile([128, CT, N], BF16, name=s + "xT")
            o_catT, free_ocat = tc.tile([128, CT, N], BF16, name=s + "o_catT")

            with (
                tc.tile_pool(name=s + "psA", bufs=2, space="PSUM") as psA,
                tc.tile_pool(name=s + "psB", bufs=1, space="PSUM") as psB,
                tc.tile_pool(name=s + "ps_o", bufs=1, space="PSUM") as ps_o_pool,
                tc.tile_pool(name=s + "wqk_st", bufs=2) as wqk_st_pool,
                tc.tile_pool(name=s + "wqk", bufs=2) as wqk_pool,
                tc.tile_pool(name=s + "stg", bufs=1) as stg_pool,
                tc.tile_pool(name=s + "qkT", bufs=2) as qkT_pool,
                tc.tile_pool(name=s + "pT", bufs=16) as pT_pool,
                tc.tile_pool(name=s + "o_raw", bufs=2) as o_raw_pool,
                tc.tile_pool(name=s + "l", bufs=2) as l_pool,
                tc.tile_pool(name=s + "y", bufs=2) as y_pool,
            ):

                def slotA(name):
                    return psA.tile([128, N], F32, name=s + name, tag="A")

                def slotB(name):
                    return psB.tile([128, N], F32, name=s + name, tag="Bacc")

                # q/k weight strips: DMA fp32 stage -> DVE cast -> bf16 strip
                def emit_wqk_dma(pj, queue):
                    stage = wqk_st_pool.tile(
                        [128, CT, 256], F32, name=f"{s}wqs{pj}", tag="wqk_st"
                    )
                    queue.dma_start(
                        out=stage[:, :, 0:128],
                        in_=wqkv_t[:, :, pj * 128 : (pj + 1) * 128],
                    )
                    queue.dma_start(
                        out=stage[:, :, 128:256],
                        in_=wqkv_t[:, :, C + pj * 128 : C + (pj + 1) * 128],
                    )
                    return stage

                def emit_wqk_cast(stage, pj):
                    strip = wqk_pool.tile(
                        [128, CT, 256], BF16, name=f"{s}wqk{pj}", tag="wqk"
                    )
                    nc.vector.tensor_copy(out=strip, in_=stage)
                    return strip

                # ---------- phase 0: x -> xT (per-tile DMA/transpose/copy) ----
                wv, free_wv = tc.tile([128, CT, C], BF16, name=s + "wv")
                with tc.tile_pool(name=s + "x_pool", bufs=3) as x_pool:
                    x_src = x_ap.rearrange("(i p) c -> p i c", p=128)
                    for i in range(NT):
                        x_i = x_pool.tile(
                            [128, C], F32, name=f"{s}x{i}", tag="x"
                        )
                        nc.sync.dma_start(out=x_i, in_=x_src[:, i, :])
                        ps_t = slotA(f"ps_t{i}")
                        for j in range(CT):
                            nc.tensor.matmul(
                                ps_t[:, j * 128 : (j + 1) * 128],
                                x_i[:, j * 128 : (j + 1) * 128],
                                identity,
                                is_transpose=True,
                            )
                        nc.vector.tensor_copy(
                            out=xT[:, :, i * 128 : (i + 1) * 128],
                            in_=ps_t.rearrange("p (j n) -> p j n", n=128),
                        )

                    # v weights: DMA fp32 halves -> ACT cast (ACT is idle here)
                    for ch in range(2):
                        csl = slice(2 * C + ch * 512, 2 * C + ch * 512 + 512)
                        wv_st = stg_pool.tile(
                            [128, CT, 512], F32, name=f"{s}wvst{ch}", tag="stg"
                        )
                        nc.gpsimd.dma_start(out=wv_st, in_=wqkv_t[:, :, csl])
                        nc.scalar.copy(
                            out=wv[:, :, ch * 512 : ch * 512 + 512], in_=wv_st
                        )

                def qk_mm_chunk(slot, strip, qk, ch):
                    nsl = slice(ch * 512, ch * 512 + 512)
                    for kt in range(CT):
                        nc.tensor.matmul(
                            slot[:, nsl],
                            strip[:, kt, qk * 128 : qk * 128 + 128],
                            xT[:, kt, nsl],
                            start=(kt == 0),
                            stop=(kt == CT - 1),
                        )

                def qk_drain(slot, qkT, pj, qk):
                    jj = qk * CT + pj
                    nc.vector.tensor_scalar_add(
                        out=qkT[:, qk, :], in0=slot, scalar1=bq_pp[:, jj : jj + 1]
                    )

                def emit_qk_full(pj, strip):
                    qkT = qkT_pool.tile(
                        [128, 2, N], BF16, name=f"{s}qkT{pj}", tag="qkT"
                    )
                    for qk in range(2):
                        slot = slotB(f"qk{pj}_{qk}")
                        for ch in range(2):
                            qk_mm_chunk(slot, strip, qk, ch)
                        qk_drain(slot, qkT, pj, qk)
                    return qkT

                def emit_v_chunk(m):
                    """Project v for token tile m (all 16 heads) -> v_ext[:, m]."""
                    slot = slotB(f"v{m}")
                    for ch in range(2):
                        nsl = slice(ch * 512, ch * 512 + 512)
                        for kt in range(CT):
                            nc.tensor.matmul(
                                slot[:, nsl],
                                xT[:, kt, m * 128 : (m + 1) * 128],
                                wv[:, kt, nsl],
                                start=(kt == 0),
                                stop=(kt == CT - 1),
                            )
                    nc.vector.tensor_add(
                        out=v_ext[:, m, :, 0:D],
                        in0=slot.rearrange("p (h d) -> p h d", d=D),
                        in1=bv_bc.rearrange("p (h d) -> p h d", d=D),
                    )

                def emit_pv(ps_o, h, kt, pT):
                    for ch in range(2):
                        nsl = slice(ch * 512, ch * 512 + 512)
                        nc.tensor.matmul(
                            ps_o[:, nsl],
                            v_ext[:, kt, h, :],
                            pT[:, nsl],
                            start=(kt == 0),
                            stop=(kt == NT - 1),
                        )

                def normalize(h, ps_o):
                    """ps_o [D+1, N] -> o_catT[hb:hb+64, pj, :] (normalized)."""
                    pj, hb = h // 2, (h % 2) * 64
                    o_raw = o_raw_pool.tile(
                        [D, N], F32, name=f"{s}o_raw{h}", tag="o_raw"
                    )
                    nc.vector.tensor_copy(out=o_raw, in_=ps_o[0:D, :])
                    l_den = l_pool.tile([1, N], F32, name=f"{s}l_den{h}", tag="l_den")
                    nc.vector.tensor_copy(out=l_den, in_=ps_o[D : D + 1, :])
                    l_inv = l_pool.tile([1, N], F32, name=f"{s}l_inv{h}", tag="l_inv")
                    nc.vector.reciprocal_approx_fast(out=l_inv, in_=l_den)
                    l_bc = l_pool.tile([D, N], F32, name=f"{s}l_bc{h}", tag="l_bc")
                    nc.gpsimd.partition_broadcast(l_bc, l_inv)
                    nc.vector.tensor_mul(
                        out=o_catT[hb : hb + 64, pj, :],
                        in0=o_raw,
                        in1=l_bc,
                    )

                def emit_sc_exp(p, kt, qkT):
                    """Scores + exps for both heads of pair p at token tile kt."""
                    h0, h1 = 2 * p, 2 * p + 1
                    ksl = slice(kt * 128, (kt + 1) * 128)
                    sc0 = slotA(f"sc{h0}_{kt}")
                    sc1 = slotA(f"sc{h1}_{kt}")
                    for ch in range(2):
                        nsl = slice(ch * 512, ch * 512 + 512)
                        nc.tensor.matmul(
                            sc0[:, nsl], qkT[0:64, 1, ksl], qkT[0:64, 0, nsl]
                        )
                        nc.tensor.matmul(
                            sc1[:, nsl], qkT[64:128, 1, ksl], qkT[64:128, 0, nsl]
                        )
                    pT0 = pT_pool.tile(
                        [128, N], BF16, name=f"{s}pT{h0}_{kt}", tag="pT"
                    )
                    nc.scalar.activation(out=pT0, in_=sc0, func=AF.Exp, scale=SCALE)
                    pT1 = pT_pool.tile(
                        [128, N], BF16, name=f"{s}pT{h1}_{kt}", tag="pT"
                    )
                    nc.scalar.activation(out=pT1, in_=sc1, func=AF.Exp, scale=SCALE)
                    return pT0, pT1

                # ---------- prologue ----------
                stage0 = emit_wqk_dma(0, nc.gpsimd)
                stage1 = emit_wqk_dma(1, nc.gpsimd)
                strip_next = emit_wqk_cast(stage0, 0)
                qkT_this = emit_qk_full(0, strip_next)
                strip_next = emit_wqk_cast(stage1, 1)
                emit_v_chunk(0)
                hoisted = emit_sc_exp(0, 0, qkT_this)
                wproj = None
                free_wproj = None

                # ---------- main pair loop ----------
                for p in range(HP):
                    h0, h1 = 2 * p, 2 * p + 1
                    ps_o = ps_o_pool.tile(
                        [D + 1, N], F32, name=f"{s}ps_o{h0}", tag="ps_o"
                    )
                    # next-pair qk slot (filled chunk-by-chunk inside kt loop)
                    qk_slot = None
                    qkT_next = None
                    if 0 < p < HP - 1:
                        qkT_next = qkT_pool.tile(
                            [128, 2, N], BF16, name=f"{s}qkT{p + 1}", tag="qkT"
                        )
                    pT_prev, pT1_0 = hoisted
                    pT_h1_tiles = [pT1_0]
                    pT1_prev = pT1_0
                    ps_o1_last = None
                    if p == HP - 1:
                        ps_o1_last = psB.tile(
                            [128, N], F32, name=f"{s}ps_o_last", tag="Bacc"
                        )[0 : D + 1, :]
                    for kt in range(1, NT):
                        if p == 0:
                            emit_v_chunk(kt)
                        elif qkT_next is not None and kt % 2 == 1:
                            # one quarter of next pair's q/k projection
                            step = kt // 2  # 0..3 -> (q,ch0),(q,ch1),(k,ch0),(k,ch1)
                            qk, ch = step // 2, step % 2
                            if ch == 0:
                                qk_slot = slotB(f"qk{p + 1}_{qk}")
                            qk_mm_chunk(qk_slot, strip_next, qk, ch)
                            if ch == 1:
                                qk_drain(qk_slot, qkT_next, p + 1, qk)
                        # pv for even head, one iteration behind the exps
                        emit_pv(ps_o, h0, kt - 1, pT_prev)
                        if p == HP - 1:
                            emit_pv(ps_o1_last, h1, kt - 1, pT1_prev)
                        pT0, pT1 = emit_sc_exp(p, kt, qkT_this)
                        pT_h1_tiles.append(pT1)
                        pT_prev = pT0
                        pT1_prev = pT1
                    emit_pv(ps_o, h0, NT - 1, pT_prev)
                    if p == HP - 1:
                        emit_pv(ps_o1_last, h1, NT - 1, pT1_prev)
                    normalize(h0, ps_o)

                    if p == 0:
                        # pair 0 had v-projection as filler; qk(1) goes here
                        qkT_next = emit_qk_full(1, strip_next)
                    if p + 2 < HP:
                        stage = emit_wqk_dma(
                            p + 2, nc.sync if p % 2 == 0 else nc.gpsimd
                        )
                        strip_next = emit_wqk_cast(stage, p + 2)

                    # hoist next pair's first scores+exps so ACT works
                    # through the pv_h1 boundary block below
                    if p + 1 < HP:
                        hoisted = emit_sc_exp(p + 1, 0, qkT_next)

                    if p == HP - 1:
                        normalize(h1, ps_o1_last)
                    else:
                        # pv for odd head from retained pT tiles (kt-outer
                        # order releases each pT tile after its two matmuls)
                        ps_o1 = ps_o_pool.tile(
                            [D + 1, N], F32, name=f"{s}ps_o{h1}", tag="ps_o"
                        )
                        for kt in range(NT):
                            emit_pv(ps_o1, h1, kt, pT_h1_tiles[kt])
                        normalize(h1, ps_o1)

                    if p == 0:
                        free_wv()
                        wproj, free_wproj = tc.tile(
                            [128, CT, C], BF16, name=s + "wproj"
                        )
                    if p in (1, 2):
                        # w_proj: DMA fp32 half -> DVE cast (mid-attention)
                        ch = p - 1
                        wp_st = stg_pool.tile(
                            [128, CT, 512], F32, name=f"{s}wpst{ch}", tag="stg"
                        )
                        nc.gpsimd.dma_start(
                            out=wp_st, in_=wproj_src[:, :, ch * 512 : ch * 512 + 512]
                        )
                        nc.vector.tensor_copy(
                            out=wproj[:, :, ch * 512 : ch * 512 + 512], in_=wp_st
                        )

                    qkT_this = qkT_next

                # ---------- output projection ----------
                for m in range(NT):
                    slot = slotA(f"ym{m}") if m % 2 == 0 else slotB(f"ym{m}")
                    for ch in range(2):
                        nsl = slice(ch * 512, ch * 512 + 512)
                        for pj in range(CT):
                            nc.tensor.matmul(
                                slot[:, nsl],
                                o_catT[:, pj, m * 128 : (m + 1) * 128],
                                wproj[:, pj, nsl],
                                start=(pj == 0),
                                stop=(pj == CT - 1),
                            )
                    y_sb = y_pool.tile([128, C], F32, name=f"{s}y{m}", tag="y")
                    nc.vector.tensor_add(out=y_sb, in0=slot, in1=bp_bc)
                    nc.gpsimd.dma_start(
                        out=out_ap[m * 128 : (m + 1) * 128, :], in_=y_sb
                    )

                free_wproj()

            free_ocat()
            free_xT()

        _free_bp()
        _free_bv()
        _free_bq()
        _free_vext()
        _free_idb()
        _free_id()

    nc.compile()
    return nc


def get_program(repeat: int = 1, max_phase: int = 3):
    key = ("nc", repeat, max_phase)
    if key not in _CACHE:
        _CACHE[key] = _build_program(repeat, max_phase)
    return _CACHE[key]


def _get_runner():
    """Persistent jitted SPMD executor (avoids re-tracing per kernel() call)."""
    if "runner" in _CACHE:
        return _CACHE["runner"]

    import jax
    from jax.sharding import Mesh, PartitionSpec
    from jax.experimental.shard_map import shard_map
    import concourse.mybir as mybir
    from concourse.bass2jax import (
        _bass_exec_p,
        install_neuronx_cc_hook,
        partition_id_tensor,
    )

    nc = get_program()
    install_neuronx_cc_hook()
    partition_name = nc.partition_id_tensor.name if nc.partition_id_tensor else None

    in_names, out_names, out_avals, zero_outs = [], [], [], []
    for alloc in nc.m.functions[0].allocations:
        if not isinstance(alloc, mybir.MemoryLocationSet):
            continue
        name = alloc.memorylocations[0].name
        if alloc.kind == "ExternalInput":
            if name != partition_name:
                in_names.append(name)
        elif alloc.kind == "ExternalOutput":
            shape = tuple(alloc.tensor_shape)
            dtype = mybir.dt.np(alloc.dtype)
            out_names.append(name)
            out_avals.append(jax.core.ShapedArray(shape, dtype))
            zero_outs.append(np.zeros((B * shape[0], *shape[1:]), dtype))
    n_params = len(in_names)
    in_names_all = list(in_names) + list(out_names)
    if partition_name is not None:
        in_names_all.append(partition_name)

    def _body(*args):
        operands = list(args)
        if partition_name is not None:
            operands.append(partition_id_tensor())
        return tuple(
            _bass_exec_p.bind(
                *operands,
                out_avals=tuple(out_avals),
                in_names=tuple(in_names_all),
                out_names=tuple(out_names),
                lowering_input_output_aliases=(),
                sim_require_finite=True,
                sim_require_nnan=True,
                nc=nc,
            )
        )

    devices = jax.devices()[:B]
    mesh = Mesh(np.asarray(devices), ("core",))
    n_outs = len(out_avals)
    sharded = jax.jit(
        shard_map(
            _body,
            mesh=mesh,
            in_specs=(PartitionSpec("core"),) * (n_params + n_outs),
            out_specs=(PartitionSpec("core"),) * n_outs,
            check_rep=False,
        ),
        keep_unused=True,
    )

    sharding = jax.sharding.NamedSharding(mesh, PartitionSpec("core"))
    dev_cache: dict = {}

    def _to_device(name, concat):
        import hashlib

        digest = hashlib.blake2b(concat.tobytes(), digest_size=16).digest()
        hit = dev_cache.get(name)
        if hit is not None and hit[0] == digest:
            return hit[1]
        arr = jax.device_put(concat, sharding)
        dev_cache[name] = (digest, arr)
        return arr

    def run(in_maps):
        concat_in = [
            _to_device(
                name,
                np.concatenate([np.asarray(m[name]) for m in in_maps], axis=0),
            )
            for name in in_names
        ]
        outs = sharded(*concat_in, *zero_outs)
        return {
            name: np.asarray(outs[i]).reshape(B, *out_avals[i].shape)
            for i, name in enumerate(out_names)
        }

    _CACHE["runner"] = run
    return run


def kernel(x, w_qkv, b_qkv, w_proj, b_proj):
    x = np.ascontiguousarray(np.asarray(x, dtype=np.float32))
    shared = {
        "w_qkv": np.ascontiguousarray(np.asarray(w_qkv, dtype=np.float32)),
        "b_qkv": np.ascontiguousarray(np.asarray(b_qkv, dtype=np.float32)),
        "w_proj": np.ascontiguousarray(np.asarray(w_proj, dtype=np.float32)),
        "b_proj": np.ascontiguousarray(np.asarray(b_proj, dtype=np.float32)),
    }
    in_maps = [{"x": x[b], **shared} for b in range(B)]
    run = _get_runner()
    res = run(in_maps)
    return res["out"].astype(np.float32)
